# revision 27
# baseline (speedup 1.0000x reference)
"""Trainium2 Bass kernel for nn_CausalSelfAttention_10368051052888.

Head-sharded tensor parallel over 8 NeuronCores (2 heads/core).
Feature-major ("transposed") layout on device: activations live as
[feature, seq] so the PE contraction dim is always the partition dim.

Schedule (single fused pipeline, PE-bound at ~1.95GHz sustained):
  1. q+k projections for all 6 seq slices in one streamed pass over xT,
     with sum-of-squares partials; chunked ssq AllReduces fire after
     slices 1/3/5 so rmsnorm scales are ready early.
  2. Per slice j: v-projection, RoPE (bf16 tables, DVE), attention for
     both heads (exp on ACT is the only Scalar-engine user from here on),
     per-(j,h) AllGather of normalized outputs, and the previous slice's
     output projection — all interleaved so ACT/DVE/CC hide under PE.

Per core c (heads 2c, 2c+1):
  - attention scores in [k, q] orientation, exp without max-subtraction
    (max|s| ~ 6.5), denominators via bf16 group accumulation on DVE +
    one GpSimd partition reduce per (j, head)
  - AllGather of normalized attention outputs (bf16), then column-sharded
    output projection y[:, c*256:(c+1)*256]

Self-contained: hardcodes the problem shapes from the spec.
"""
import numpy as np
import ml_dtypes

import concourse.bass as bass
import concourse.bass_isa as bass_isa
import concourse.mybir as mybir
import concourse.tile as tile
from concourse import bacc
from concourse.bass_utils import run_bass_kernel_spmd

BF = ml_dtypes.bfloat16

N_CORES = 8
S = 2640
D = 2048
H = 16
HD = 128
CACHE = 5280
EPS = 1e-6

HPC = H // N_CORES          # heads per core = 2
MF = HPC * HD               # per-core feature slice = 256
L = CACHE + S               # 7920
KC = D // 128               # 16 contraction chunks
CTILES = (CACHE + 127) // 128   # 42 cache k-tiles (last kt=32)
NTILES = (S + 127) // 128       # 21 new k-tiles / v s-tiles (last 80)
VPAD = CTILES * 128             # 5376 padded cache rows for v
NQ = 512
N_SLICES = [(i * NQ, min(NQ, S - i * NQ)) for i in range((S + NQ - 1) // NQ)]
NJ = len(N_SLICES)
# ssq AllReduce chunks: slice 0 alone (fires earliest, its latency gates
# the attention start), then (1,2), then (3,4,5)
AR_CHUNKS = [(0, 1), (1, 3), (3, 6)]
# recv_ar(ci) is emitted just before the first rope slice that needs it
AR_RECV_AT = {0: 0, 1: 1, 2: 3}

SWAP_MASK = [(i ^ 1) for i in range(32)]  # pair swap within 32-partition groups

# feature index for each per-head-AllGather output row r = h*1024 + core*128 + p
AG_PERM = np.array([(r % 1024) // 128 * MF + (r // 1024) * 128 + (r % 128)
                    for r in range(D)])

_prog_cache = {}


def build_program():
    dt = mybir.dt
    f32, bf16 = dt.float32, dt.bfloat16
    nc = bacc.Bacc("TRN2", target_bir_lowering=False, debug=False,
                   num_devices=N_CORES)

    # ---------------- I/O ----------------
    xT = nc.dram_tensor("xT", [D, S], bf16, kind="ExternalInput")
    wq = nc.dram_tensor("wq", [128, KC * MF], bf16, kind="ExternalInput")
    wk = nc.dram_tensor("wk", [128, KC * MF], bf16, kind="ExternalInput")
    wv = nc.dram_tensor("wv", [128, KC * MF], bf16, kind="ExternalInput")
    wo = nc.dram_tensor("wo", [128, KC * MF], bf16, kind="ExternalInput")
    cosT = nc.dram_tensor("cosT", [128, S], bf16, kind="ExternalInput")
    sinT = nc.dram_tensor("sinT", [128, S], bf16, kind="ExternalInput")
    kTc = nc.dram_tensor("kTc", [HPC, 128, CACHE], bf16, kind="ExternalInput")
    vc = nc.dram_tensor("vc", [HPC, 128, VPAD], bf16, kind="ExternalInput")
    masks = nc.dram_tensor("masks", [4, 128, NQ], bf16, kind="ExternalInput")
    yT = nc.dram_tensor("yT", [MF, S], f32, kind="ExternalOutput")

    # chunked ssq reduction buffers: [1, 2*w] packing [q-chunk | k-chunk].
    # Cross-core reduction is an AllGather of partials + a local 8-row
    # GpSimd reduce — much lower latency than a tiny AllReduce (~12µs vs
    # ~45µs of ncfw control plane after the rendezvous).
    ssq_in_d = []
    ssq_out_d = []
    for ci, (j0, j1) in enumerate(AR_CHUNKS):
        w = N_SLICES[j1 - 1][0] + N_SLICES[j1 - 1][1] - N_SLICES[j0][0]
        ssq_in_d.append(nc.dram_tensor(f"ssq_in{ci}", [1, 2 * w], f32))
        ssq_out_d.append(nc.dram_tensor(f"ssq_out{ci}", [N_CORES, 2 * w],
                                        f32, addr_space="Shared"))
    ag_in = [nc.dram_tensor(f"ag_in{j}", [HPC, 128, nn], bf16)
             for j, (qb, nn) in enumerate(N_SLICES)]
    ag_out = [nc.dram_tensor(f"ag_out{j}", [HPC, N_CORES * 128, nn], bf16,
                             addr_space="Shared")
              for j, (qb, nn) in enumerate(N_SLICES)]

    RG = [list(range(N_CORES))]
    Exp = mybir.ActivationFunctionType.Exp
    Sqrt = mybir.ActivationFunctionType.Sqrt
    Square = mybir.ActivationFunctionType.Square
    add_op = mybir.AluOpType.add
    mult_op = mybir.AluOpType.mult

    with tile.TileContext(nc) as tc:
        with (
            tc.tile_pool(name="const", bufs=1) as constp,
            tc.tile_pool(name="xs", bufs=4) as xsp,
            tc.tile_pool(name="work", bufs=2) as workp,
            tc.tile_pool(name="ftmp", bufs=3) as ftmp,
            tc.tile_pool(name="attn", bufs=3) as attnp,
            tc.tile_pool(name="ptp", bufs=3) as ptp,
            tc.tile_pool(name="psac", bufs=4, space="PSUM") as psac,
            tc.tile_pool(name="pssc", bufs=2, space="PSUM") as pssc,
        ):
            # ------------ persistent SBUF + prologue DMAs ------------
            w_sb = {}
            for name in ("q", "k", "v", "o"):
                w_sb[name] = constp.tile([128, KC * MF], bf16,
                                         tag=f"w{name}", name=f"w{name}")
            # q/k weights on the (idle) scalar queue so the x stream owns
            # the sync queue from t=0; bulk cache loads go via gpsimd SWDGE
            nc.scalar.dma_start(out=w_sb["q"][:], in_=wq[:])
            nc.scalar.dma_start(out=w_sb["k"][:], in_=wk[:])
            cos_sb = constp.tile([128, S], bf16, tag="cos")
            sin_sb = constp.tile([128, S], bf16, tag="sin")
            mask_sb = constp.tile([128, 4 * NQ], bf16, tag="masks")
            kT_sb = []
            v_sb = []
            for h in range(HPC):
                kt_t = constp.tile([128, L], bf16, tag=f"kT{h}", name=f"kT{h}")
                kT_sb.append(kt_t)
                v_t = constp.tile([128, VPAD + NTILES * 128], bf16,
                                  tag=f"v{h}", name=f"vsb{h}")
                v_sb.append(v_t)

            def load_bulk_first():
                # head-0 cache only, deferred past slice 0 so the startup
                # HBM bandwidth goes to wq/wk + the x stream; scalar HWDGE
                # queue carries no compute so ring waits are harmless
                nc.scalar.dma_start(out=kT_sb[0][:, :CACHE], in_=kTc[0])
                nc.scalar.dma_start(out=v_sb[0][:, :VPAD], in_=vc[0])

            def load_bulk_rest():
                # remaining constants land during vproj(0)/attn(0,0)
                nc.scalar.dma_start(out=cos_sb[:], in_=cosT[:])
                nc.scalar.dma_start(out=sin_sb[:], in_=sinT[:])
                nc.scalar.dma_start(
                    out=mask_sb[:].rearrange("p (d c) -> p d c", c=NQ),
                    in_=masks[:].rearrange("d p c -> p d c"),
                )
                nc.scalar.dma_start(out=kT_sb[1][:, :CACHE], in_=kTc[1])
                nc.scalar.dma_start(out=v_sb[1][:, :VPAD], in_=vc[1])
            # rq_sb doubles as the q staging buffer (rope runs in place);
            # k stages directly into kT_sb[:, CACHE:].
            rq_sb = [constp.tile([128, S], bf16, tag=f"rq{h}", name=f"rq{h}")
                     for h in range(HPC)]
            onescol = constp.tile([128, 1], bf16, tag="onescol")
            nc.vector.memset(onescol[:], 1.0)
            # per-AR-chunk ssq partial tiles (separate tiles so a chunk's
            # readback doesn't pick up false whole-tile deps on later slices)
            ssq_c = []
            for ci, (j0, j1) in enumerate(AR_CHUNKS):
                w = (N_SLICES[j1 - 1][0] + N_SLICES[j1 - 1][1]
                     - N_SLICES[j0][0])
                ssq_c.append([
                    constp.tile([1, w], f32, tag=f"ssq_c{ci}_{ti}",
                                name=f"ssq_c{ci}_{ti}")
                    for ti in range(2)])
            chunk_of = {}
            for ci, (j0, j1) in enumerate(AR_CHUNKS):
                for j in range(j0, j1):
                    chunk_of[j] = ci
            # bf16 rsqrt rows used by rope (q row 0 / k row 0 separately)
            srow16 = [constp.tile([1, S], bf16, tag=f"srow16_{i}",
                                  name=f"srow16_{i}")
                      for i in range(2)]
            eps_col = constp.tile([1, 1], f32, tag="eps")
            nc.vector.memset(eps_col[:], EPS)

            def stage_dest(tname, m, qb, nn):
                if tname == "q":
                    return rq_sb[m][:, qb:qb + nn]
                return kT_sb[m][:, CACHE + qb:CACHE + qb + nn]

            def stream_x(qb, nn, consume):
                """DMA xT[:, qb:qb+nn] in 4-chunk groups; call consume(kc, rhs_ap)."""
                for g in range(KC // 4):
                    xs = xsp.tile([128, 4 * NQ], bf16, tag="xs", name="xs")
                    nc.sync.dma_start(
                        out=xs[:].rearrange("p (a n) -> p a n", n=NQ)[:, :, :nn],
                        in_=xT[g * 512:(g + 1) * 512, qb:qb + nn]
                            .rearrange("(a p) n -> p a n", p=128))
                    for kcl in range(4):
                        consume(g * 4 + kcl, xs[:, kcl * NQ:kcl * NQ + nn])

            def chunk_cols(ci):
                j0, j1 = AR_CHUNKS[ci]
                c0 = N_SLICES[j0][0]
                w = N_SLICES[j1 - 1][0] + N_SLICES[j1 - 1][1] - c0
                return c0, w, slice(c0, c0 + w)

            def fire_ar(ci):
                c0, w, sl = chunk_cols(ci)
                nc.gpsimd.dma_start(out=ssq_in_d[ci][:, :w],
                                    in_=ssq_c[ci][0][:])
                nc.gpsimd.dma_start(out=ssq_in_d[ci][:, w:],
                                    in_=ssq_c[ci][1][:])
                nc.gpsimd.collective_compute(
                    "AllGather", mybir.AluOpType.bypass, replica_groups=RG,
                    ins=[ssq_in_d[ci][:]], outs=[ssq_out_d[ci][:]])

            def recv_ar(ci):
                c0, w, sl = chunk_cols(ci)
                # gather-readback + local 8-row reduce, in <=1024-col pieces
                for ti in range(2):
                    t_ = ssq_c[ci][ti]
                    for p0 in range(0, w, 2 * NQ):
                        pw = min(2 * NQ, w - p0)
                        parts = workp.tile([N_CORES, 2 * NQ], f32,
                                           tag="ssqparts", bufs=1,
                                           name="ssq_parts")
                        nc.gpsimd.dma_start(
                            out=parts[:, :pw],
                            in_=ssq_out_d[ci][:, ti * w + p0:ti * w + p0 + pw])
                        nc.gpsimd.partition_all_reduce(
                            parts[:, :pw], parts[:, :pw], channels=N_CORES,
                            reduce_op=bass_isa.ReduceOp.add)
                        # srow = 1/sqrt(ssq/D + eps), then bf16 for rope
                        nc.scalar.activation(t_[:, p0:p0 + pw],
                                             parts[:1, :pw], Sqrt,
                                             scale=1.0 / D, bias=eps_col[:])
                    nc.vector.reciprocal_approx_fast(out=t_[:], in_=t_[:])
                    nc.vector.tensor_copy(srow16[ti][:, sl], t_[:])

            # ---- phase 1: merged q+k projection, all slices ----
            for (qb, nn) in N_SLICES:
                pst = {t: [psac.tile([128, NQ], f32, tag="acc",
                                     name=f"proj_{t}{m}")
                           for m in range(HPC)] for t in ("q", "k")}

                def mm_proj(kc, rhs, pst=pst, nn=nn):
                    for t in ("q", "k"):
                        for m in range(HPC):
                            nc.tensor.matmul(
                                pst[t][m][:, :nn],
                                w_sb[t][:, kc * MF + m * 128:
                                        kc * MF + (m + 1) * 128],
                                rhs, start=(kc == 0), stop=(kc == KC - 1))

                stream_x(qb, nn, mm_proj)
                # ssq partials: stage bf16 (DVE), square the staged copy
                # (DVE, keeps the scalar queue free for DMA), ones-matmul
                # reduce on PE
                sqp = pssc.tile([128, 2 * NQ], f32, tag="scores", name="sqp")
                for ti, t in enumerate(("q", "k")):
                    for m in range(HPC):
                        # stage raw q/k as bf16 for post-AR in-place rope
                        st = stage_dest(t, m, qb, nn)
                        nc.vector.tensor_copy(st, pst[t][m][:, :nn])
                        q2 = workp.tile([128, NQ], bf16, tag="btmp")
                        nc.vector.tensor_tensor(q2[:, :nn], st, st, mult_op)
                        nc.tensor.matmul(sqp[:1, ti * NQ:ti * NQ + nn],
                                         onescol[:], q2[:, :nn],
                                         start=(m == 0), stop=(m == HPC - 1))
                    ci = chunk_of[qb // NQ]
                    lo = qb - N_SLICES[AR_CHUNKS[ci][0]][0]
                    nc.vector.tensor_copy(ssq_c[ci][ti][:, lo:lo + nn],
                                          sqp[:1, ti * NQ:ti * NQ + nn])
                # fire the AllReduce as soon as its last slice is done
                for ci, (j0, j1) in enumerate(AR_CHUNKS):
                    if qb == N_SLICES[j1 - 1][0]:
                        fire_ar(ci)
                        if ci == 0:
                            load_bulk_first()
            load_bulk_rest()
            nc.scalar.dma_start(out=w_sb["v"][:], in_=wv[:])

            # ---------------- per-slice helpers ----------------
            def rope_j(j):
                qb, nn = N_SLICES[j]
                for m in range(HPC):
                    for ti, tname in enumerate(("q", "k")):
                        st = stage_dest(tname, m, qb, nn)
                        sh = workp.tile([128, NQ], bf16, tag="btmp")
                        nc.vector.stream_shuffle(sh[:, :nn], st, SWAP_MASK)
                        a = ftmp.tile([128, NQ], bf16, tag="btmp2",
                                      name="rope_a")
                        nc.vector.tensor_tensor(
                            a[:, :nn], st, cos_sb[:, qb:qb + nn], mult_op)
                        b = ftmp.tile([128, NQ], bf16, tag="btmp2",
                                      name="rope_b")
                        nc.vector.tensor_tensor(
                            b[:, :nn], sh[:, :nn], sin_sb[:, qb:qb + nn],
                            mult_op)
                        nc.vector.tensor_tensor(a[:, :nn], a[:, :nn],
                                                b[:, :nn], add_op)
                        srb = workp.tile([128, NQ], bf16, tag="srowb")
                        nc.gpsimd.partition_broadcast(
                            srb[:, :nn], srow16[ti][:, qb:qb + nn])
                        nc.vector.tensor_tensor(st, a[:, :nn], srb[:, :nn],
                                                mult_op)

            def vproj_j(j):
                qb, nn = N_SLICES[j]
                nst = (nn + 127) // 128
                xsg = []
                for g in range(KC // 4):
                    xs = xsp.tile([128, 4 * NQ], bf16, tag="xs", name="xsv")
                    nc.sync.dma_start(
                        out=xs[:].rearrange("p (a n) -> p a n", n=NQ)[:, :, :nn],
                        in_=xT[g * 512:(g + 1) * 512, qb:qb + nn]
                            .rearrange("(a p) n -> p a n", p=128))
                    xsg.append(xs)
                for s_ in range(nst):
                    sw = min(128, nn - s_ * 128)
                    pv = psac.tile([128, NQ], f32, tag="acc", name="pv_ps")
                    for g in range(KC // 4):
                        for kcl in range(4):
                            kc = g * 4 + kcl
                            nc.tensor.matmul(
                                pv[:sw, :MF],
                                xsg[g][:, kcl * NQ + s_ * 128:
                                       kcl * NQ + s_ * 128 + sw],
                                w_sb["v"][:, kc * MF:(kc + 1) * MF],
                                start=(kc == 0), stop=(kc == KC - 1))
                    st_glob = (qb + s_ * 128) // 128
                    for h in range(HPC):
                        nc.vector.tensor_copy(
                            v_sb[h][:sw, VPAD + st_glob * 128:
                                    VPAD + st_glob * 128 + 128],
                            pv[:sw, h * 128:(h + 1) * 128])

            scale = float(HD) ** -0.5
            GSZ = 8   # pairs per bf16 partial-sum group (16 k-tiles)

            def yproj(j):
                qb, nn = N_SLICES[j]
                py = [psac.tile([128, NQ], f32, tag="acc", name="py_ps")
                      for _ in range(HPC)]
                for g in range(KC // 4):
                    gt = xsp.tile([128, 4 * NQ], bf16, tag="xs", name="gt")
                    nc.sync.dma_start(
                        out=gt[:].rearrange("p (a n) -> p a n", n=NQ)[:, :, :nn],
                        in_=ag_out[j].rearrange("h r n -> (h r) n")
                            [g * 512:(g + 1) * 512, :]
                            .rearrange("(a p) n -> p a n", p=128))
                    for kcl in range(4):
                        kc = g * 4 + kcl
                        for m in range(HPC):
                            nc.tensor.matmul(
                                py[m][:, :nn],
                                w_sb["o"][:, kc * MF + m * 128:
                                          kc * MF + (m + 1) * 128],
                                gt[:, kcl * NQ:kcl * NQ + nn],
                                start=(kc == 0), stop=(kc == KC - 1))
                for m in range(HPC):
                    ys = ftmp.tile([128, NQ], f32, tag="f32tmp", name="ys")
                    nc.vector.tensor_copy(ys[:, :nn], py[m][:, :nn])
                    nc.sync.dma_start(
                        out=yT[m * 128:(m + 1) * 128, qb:qb + nn],
                        in_=ys[:, :nn])

            def attn_jh(j, h):
                qb, nn = N_SLICES[j]
                # k-tile list: (col0 in kT_sb, kt, vcol0, mask_off)
                tiles = []
                for ct in range(CTILES):
                    kt = min(128, CACHE - ct * 128)
                    tiles.append((ct * 128, kt, ct * 128, None))
                for t in range(NTILES):
                    kb = t * 128
                    if kb > qb + nn - 1:
                        continue
                    kt = min(128, S - kb)
                    moff = (kb - qb) if (kb + kt - 1) > qb else None
                    tiles.append((CACHE + kb, kt, VPAD + kb, moff))
                # pair up consecutive full tiles to halve per-instruction
                # overheads on ACT/DVE
                pairs = []
                i = 0
                while i < len(tiles):
                    if (i + 1 < len(tiles) and tiles[i][1] == 128
                            and tiles[i + 1][1] == 128):
                        pairs.append((tiles[i], tiles[i + 1]))
                        i += 2
                    else:
                        pairs.append((tiles[i],))
                        i += 1
                out_ps = psac.tile([128, NQ], f32, tag="acc", name="out_ps")
                pacc = attnp.tile([128, 2 * NQ], f32, tag="pacc", bufs=2)
                rq_slice = rq_sb[h][:, qb:qb + nn]
                nidx = 0
                nlast = len(tiles) - 1
                gacc = None
                gcount = 0
                pacc_init = False

                def flush(nn=nn):
                    nonlocal gacc, gcount, pacc_init
                    if gacc is None:
                        return
                    gv = gacc[:].rearrange(
                        "p (a n) -> p a n", n=NQ)[:, :, :nn]
                    pv_ = pacc[:].rearrange(
                        "p (a n) -> p a n", n=NQ)[:, :, :nn]
                    if pacc_init:
                        nc.vector.tensor_tensor(pv_, pv_, gv, add_op)
                    else:
                        nc.vector.tensor_copy(pv_, gv)
                    gacc = None
                    gcount = 0
                    pacc_init = True

                for pair in pairs:
                    full_pair = len(pair) == 2
                    sc = pssc.tile([128, 2 * NQ], f32, tag="scores")
                    for half, (c0, kt, vcol, moff) in enumerate(pair):
                        nc.tensor.matmul(
                            sc[:kt, half * NQ:half * NQ + nn],
                            kT_sb[h][:, c0:c0 + kt],
                            rq_slice, start=True, stop=True)
                    kt0 = pair[0][1]
                    # exp of a group's first full pair writes the group
                    # accumulator directly (saves a DVE copy per group)
                    new_group = full_pair and gacc is None
                    if new_group:
                        gacc = attnp.tile([128, 2 * NQ], bf16,
                                          tag="gacc", bufs=2)
                        pt = gacc
                        gcount = 1
                    else:
                        pt = ptp.tile([128, 2 * NQ], bf16, tag="pT")
                    if full_pair:
                        nc.scalar.activation(
                            pt[:].rearrange("p (a n) -> p a n",
                                            n=NQ)[:, :, :nn],
                            sc[:].rearrange("p (a n) -> p a n",
                                            n=NQ)[:, :, :nn],
                            Exp, scale=scale)
                    else:
                        nc.scalar.activation(pt[:kt0, :nn],
                                             sc[:kt0, :nn], Exp,
                                             scale=scale)
                    for half, (c0, kt, vcol, moff) in enumerate(pair):
                        if moff is not None:
                            mi = moff // 128
                            nc.vector.tensor_tensor(
                                pt[:kt, half * NQ:half * NQ + nn],
                                pt[:kt, half * NQ:half * NQ + nn],
                                mask_sb[:kt, mi * NQ:mi * NQ + nn],
                                mult_op)
                    # denominator accumulation: bf16 groups of GSZ pairs,
                    # folded into fp32 pacc; odd tiles direct
                    if full_pair:
                        if not new_group:
                            nc.vector.tensor_tensor(
                                gacc[:].rearrange("p (a n) -> p a n",
                                                  n=NQ)[:, :, :nn],
                                gacc[:].rearrange("p (a n) -> p a n",
                                                  n=NQ)[:, :, :nn],
                                pt[:].rearrange("p (a n) -> p a n",
                                                n=NQ)[:, :, :nn],
                                add_op)
                            gcount += 1
                        if gcount == GSZ:
                            flush()
                    else:
                        flush()
                        if pacc_init:
                            nc.vector.tensor_tensor(
                                pacc[:kt0, :nn], pacc[:kt0, :nn],
                                pt[:kt0, :nn], add_op)
                        else:
                            nc.vector.tensor_copy(pacc[:kt0, :nn],
                                                  pt[:kt0, :nn])
                            pacc_init = True
                    for half, (c0, kt, vcol, moff) in enumerate(pair):
                        nc.tensor.matmul(
                            out_ps[:, :nn],
                            v_sb[h][:kt, vcol:vcol + 128],
                            pt[:kt, half * NQ:half * NQ + nn],
                            start=(nidx == 0), stop=(nidx == nlast))
                        nidx += 1
                flush()
                # fold the two halves, reduce over partitions, reciprocal
                nc.vector.tensor_tensor(pacc[:, :nn], pacc[:, :nn],
                                        pacc[:, NQ:NQ + nn], add_op)
                recb = attnp.tile([128, NQ], f32, tag="recb", bufs=2)
                nc.gpsimd.partition_all_reduce(
                    recb[:, :nn], pacc[:, :nn], channels=128,
                    reduce_op=bass_isa.ReduceOp.add)
                nc.vector.reciprocal_approx_fast(out=recb[:, :nn],
                                                 in_=recb[:, :nn])
                onorm = attnp.tile([128, NQ], bf16, tag="onorm", bufs=2)
                nc.vector.tensor_tensor(onorm[:, :nn], out_ps[:, :nn],
                                        recb[:, :nn], mult_op)
                nc.gpsimd.dma_start(out=ag_in[j][h][:, :nn],
                                    in_=onorm[:, :nn])
                nc.gpsimd.collective_compute(
                    "AllGather", mybir.AluOpType.bypass, replica_groups=RG,
                    ins=[ag_in[j][h]], outs=[ag_out[j][h]])

            # ---- phase 2: per-slice vproj / rope / attention / yproj ----
            # The last two slices interleave head-wise so the final
            # AllGathers on the serial CC queue are the small (80-wide)
            # ones and the 512-wide ones overlap attention compute.
            nc.scalar.dma_start(out=w_sb["o"][:], in_=wo[:])
            recv_at = {j: ci for ci, j in AR_RECV_AT.items()}
            for j in range(NJ - 2):
                if j in recv_at:
                    recv_ar(recv_at[j])
                rope_j(j)
                vproj_j(j)
                attn_jh(j, 0)
                attn_jh(j, 1)
                if j >= 1:
                    yproj(j - 1)
            for j in (NJ - 2, NJ - 1):
                if j in recv_at:
                    recv_ar(recv_at[j])
                rope_j(j)
                vproj_j(j)
            attn_jh(NJ - 2, 0)
            yproj(NJ - 3)
            attn_jh(NJ - 2, 1)
            attn_jh(NJ - 1, 0)
            attn_jh(NJ - 1, 1)
            yproj(NJ - 2)
            yproj(NJ - 1)
    nc.compile()
    return nc


def get_program():
    if "nc" not in _prog_cache:
        _prog_cache["nc"] = build_program()
    return _prog_cache["nc"]


def prep_inputs(x, freqs, k_cache, v_cache, Wq, bq, Wk, bk, Wv, bv, Wo, bo,
                gq, gk, current_start):
    """Host-side sharding/layout. Returns per-core in_maps."""
    cs = int(current_start)
    x = np.asarray(x, dtype=np.float32)
    xT = np.ascontiguousarray(x[0].T).astype(BF)           # [D, S]
    freqs = np.asarray(freqs, dtype=np.float32)
    csl = freqs[cs:cs + S, :HD // 2]                       # [S, 64]
    snl = freqs[cs:cs + S, HD // 2:]                       # [S, 64]
    cosT = np.empty((128, S), np.float32)
    sinT = np.empty((128, S), np.float32)
    cosT[0::2] = csl.T
    cosT[1::2] = csl.T
    sinT[0::2] = -snl.T
    sinT[1::2] = snl.T
    cosT = cosT.astype(BF)
    sinT = sinT.astype(BF)
    # spec guarantees zero biases and unit gains; the device program
    # relies on that (cheap to add back via K=1 bias matmuls if needed)
    for b in (bq, bk, bv, bo):
        assert not np.any(np.asarray(b)), "nonzero bias unsupported"
    for g in (gq, gk):
        assert np.all(np.asarray(g) == 1.0), "non-unit gain unsupported"
    # masks: multiplicative {0,1}, mask_d[r, c] = 1 if c >= r + d
    masks = np.zeros((4, 128, NQ), np.float32)
    r = np.arange(128)[:, None]
    c = np.arange(NQ)[None, :]
    for di, d in enumerate((0, 128, 256, 384)):
        masks[di] = (c >= r + d).astype(np.float32)
    masks = masks.astype(BF)

    k_cache = np.asarray(k_cache, np.float32)
    v_cache = np.asarray(v_cache, np.float32)

    def wlayout(W, sl, perm=None):
        wt = np.ascontiguousarray(np.asarray(W, np.float32)[sl].T)  # [D, MF]
        if perm is not None:
            wt = wt[perm]
        return np.ascontiguousarray(
            wt.reshape(KC, 128, MF).transpose(1, 0, 2)
            .reshape(128, KC * MF)).astype(BF)

    in_maps = []
    for core in range(N_CORES):
        h0 = core * HPC
        sl = slice(core * MF, (core + 1) * MF)
        kTc = np.ascontiguousarray(
            np.transpose(k_cache[:, h0:h0 + HPC, :], (1, 2, 0))).astype(BF)
        vpad = np.zeros((HPC, 128, VPAD), BF)
        for h in range(HPC):
            vt = np.zeros((VPAD, HD), np.float32)
            vt[:CACHE] = v_cache[:, h0 + h, :]
            vpad[h] = np.ascontiguousarray(
                vt.reshape(CTILES, 128, HD).transpose(1, 0, 2)
                .reshape(128, VPAD)).astype(BF)
        in_maps.append({
            "xT": xT,
            "wq": wlayout(Wq, sl),
            "wk": wlayout(Wk, sl),
            "wv": wlayout(Wv, sl),
            "wo": wlayout(Wo, sl, perm=AG_PERM),
            "cosT": cosT,
            "sinT": sinT,
            "kTc": kTc,
            "vc": vpad,
            "masks": masks,
        })
    return in_maps


def assemble_output(results):
    cols = [np.asarray(r["yT"], np.float32).T for r in results]  # [S, MF] each
    return np.ascontiguousarray(np.concatenate(cols, axis=1))[None]


def run(inputs, trace=False):
    nc = get_program()
    in_maps = prep_inputs(**inputs)
    r = run_bass_kernel_spmd(nc, in_maps, core_ids=list(range(N_CORES)),
                             trace=trace)
    return assemble_output(r.results), r


def kernel(**inputs):
    out, _ = run(inputs, trace=False)
    return out


# revision 29
# speedup vs baseline: 1.0162x; 1.0162x over previous
"""Trainium2 Bass kernel for nn_CausalSelfAttention_10368051052888.

Head-sharded tensor parallel over 8 NeuronCores (2 heads/core).
Feature-major ("transposed") layout on device: activations live as
[feature, seq] so the PE contraction dim is always the partition dim.

Schedule (single fused pipeline, PE-bound at ~1.95GHz sustained):
  1. q+k projections for all 6 seq slices in one streamed pass over xT,
     with sum-of-squares partials; chunked ssq AllReduces fire after
     slices 1/3/5 so rmsnorm scales are ready early.
  2. Per slice j: v-projection, RoPE (bf16 tables, DVE), attention for
     both heads (exp on ACT is the only Scalar-engine user from here on),
     per-(j,h) AllGather of normalized outputs, and the previous slice's
     output projection — all interleaved so ACT/DVE/CC hide under PE.

Per core c (heads 2c, 2c+1):
  - attention scores in [k, q] orientation, exp without max-subtraction
    (max|s| ~ 6.5), denominators via bf16 group accumulation on DVE +
    one GpSimd partition reduce per (j, head)
  - AllGather of normalized attention outputs (bf16), then column-sharded
    output projection y[:, c*256:(c+1)*256]

Self-contained: hardcodes the problem shapes from the spec.
"""
import numpy as np
import ml_dtypes

import concourse.bass as bass
import concourse.bass_isa as bass_isa
import concourse.mybir as mybir
import concourse.tile as tile
from concourse import bacc
from concourse.bass_utils import run_bass_kernel_spmd

BF = ml_dtypes.bfloat16

N_CORES = 8
S = 2640
D = 2048
H = 16
HD = 128
CACHE = 5280
EPS = 1e-6

HPC = H // N_CORES          # heads per core = 2
MF = HPC * HD               # per-core feature slice = 256
L = CACHE + S               # 7920
KC = D // 128               # 16 contraction chunks
CTILES = (CACHE + 127) // 128   # 42 cache k-tiles (last kt=32)
NTILES = (S + 127) // 128       # 21 new k-tiles / v s-tiles (last 80)
VPAD = CTILES * 128             # 5376 padded cache rows for v
NQ = 512
# query slices: 128-aligned starts (mask tables assume it); the tail is
# split 384+208 instead of a pathological 80-wide slice whose N=80
# matmuls would be drain-dominated on the PE
N_SLICES = [(0, 512), (512, 512), (1024, 512), (1536, 512),
            (2048, 384), (2432, 208)]
NJ = len(N_SLICES)
# ssq AllReduce chunks: slice 0 alone (fires earliest, its latency gates
# the attention start), then (1,2), then (3,4,5)
AR_CHUNKS = [(0, 1), (1, 3), (3, 6)]
# recv_ar(ci) is emitted just before the first rope slice that needs it
AR_RECV_AT = {0: 0, 1: 1, 2: 3}

SWAP_MASK = [(i ^ 1) for i in range(32)]  # pair swap within 32-partition groups

# feature index for each per-head-AllGather output row r = h*1024 + core*128 + p
AG_PERM = np.array([(r % 1024) // 128 * MF + (r // 1024) * 128 + (r % 128)
                    for r in range(D)])

_prog_cache = {}


def build_program():
    dt = mybir.dt
    f32, bf16 = dt.float32, dt.bfloat16
    nc = bacc.Bacc("TRN2", target_bir_lowering=False, debug=False,
                   num_devices=N_CORES)

    # ---------------- I/O ----------------
    xT = nc.dram_tensor("xT", [D, S], bf16, kind="ExternalInput")
    wq = nc.dram_tensor("wq", [128, KC * MF], bf16, kind="ExternalInput")
    wk = nc.dram_tensor("wk", [128, KC * MF], bf16, kind="ExternalInput")
    wv = nc.dram_tensor("wv", [128, KC * MF], bf16, kind="ExternalInput")
    wo = nc.dram_tensor("wo", [128, KC * MF], bf16, kind="ExternalInput")
    cosT = nc.dram_tensor("cosT", [128, S], bf16, kind="ExternalInput")
    sinT = nc.dram_tensor("sinT", [128, S], bf16, kind="ExternalInput")
    kTc = nc.dram_tensor("kTc", [HPC, 128, CACHE], bf16, kind="ExternalInput")
    vc = nc.dram_tensor("vc", [HPC, 128, VPAD], bf16, kind="ExternalInput")
    masks = nc.dram_tensor("masks", [4, 128, NQ], bf16, kind="ExternalInput")
    yT = nc.dram_tensor("yT", [MF, S], f32, kind="ExternalOutput")

    # chunked ssq reduction buffers: [1, 2*w] packing [q-chunk | k-chunk].
    # Cross-core reduction is an AllGather of partials + a local 8-row
    # GpSimd reduce — much lower latency than a tiny AllReduce (~12µs vs
    # ~45µs of ncfw control plane after the rendezvous).
    ssq_in_d = []
    ssq_out_d = []
    for ci, (j0, j1) in enumerate(AR_CHUNKS):
        w = N_SLICES[j1 - 1][0] + N_SLICES[j1 - 1][1] - N_SLICES[j0][0]
        ssq_in_d.append(nc.dram_tensor(f"ssq_in{ci}", [1, 2 * w], f32))
        ssq_out_d.append(nc.dram_tensor(f"ssq_out{ci}", [N_CORES, 2 * w],
                                        f32, addr_space="Shared"))
    ag_in = [nc.dram_tensor(f"ag_in{j}", [HPC, 128, nn], bf16)
             for j, (qb, nn) in enumerate(N_SLICES)]
    ag_out = [nc.dram_tensor(f"ag_out{j}", [HPC, N_CORES * 128, nn], bf16,
                             addr_space="Shared")
              for j, (qb, nn) in enumerate(N_SLICES)]

    RG = [list(range(N_CORES))]
    Exp = mybir.ActivationFunctionType.Exp
    Sqrt = mybir.ActivationFunctionType.Sqrt
    Square = mybir.ActivationFunctionType.Square
    add_op = mybir.AluOpType.add
    mult_op = mybir.AluOpType.mult

    with tile.TileContext(nc) as tc:
        with (
            tc.tile_pool(name="const", bufs=1) as constp,
            tc.tile_pool(name="xs", bufs=4) as xsp,
            tc.tile_pool(name="work", bufs=2) as workp,
            tc.tile_pool(name="ftmp", bufs=3) as ftmp,
            tc.tile_pool(name="attn", bufs=3) as attnp,
            tc.tile_pool(name="ptp", bufs=3) as ptp,
            tc.tile_pool(name="psac", bufs=4, space="PSUM") as psac,
            tc.tile_pool(name="pssc", bufs=2, space="PSUM") as pssc,
        ):
            # ------------ persistent SBUF + prologue DMAs ------------
            w_sb = {}
            for name in ("q", "k", "v", "o"):
                w_sb[name] = constp.tile([128, KC * MF], bf16,
                                         tag=f"w{name}", name=f"w{name}")
            # q/k weights on the (idle) scalar queue so the x stream owns
            # the sync queue from t=0; bulk cache loads go via gpsimd SWDGE
            nc.scalar.dma_start(out=w_sb["q"][:], in_=wq[:])
            nc.scalar.dma_start(out=w_sb["k"][:], in_=wk[:])
            cos_sb = constp.tile([128, S], bf16, tag="cos")
            sin_sb = constp.tile([128, S], bf16, tag="sin")
            mask_sb = constp.tile([128, 4 * NQ], bf16, tag="masks")
            kT_sb = []
            v_sb = []
            for h in range(HPC):
                kt_t = constp.tile([128, L], bf16, tag=f"kT{h}", name=f"kT{h}")
                kT_sb.append(kt_t)
                v_t = constp.tile([128, VPAD + NTILES * 128], bf16,
                                  tag=f"v{h}", name=f"vsb{h}")
                v_sb.append(v_t)

            def load_bulk_first():
                # head-0 cache only, deferred past slice 0 so the startup
                # HBM bandwidth goes to wq/wk + the x stream; scalar HWDGE
                # queue carries no compute so ring waits are harmless
                nc.scalar.dma_start(out=kT_sb[0][:, :CACHE], in_=kTc[0])
                nc.scalar.dma_start(out=v_sb[0][:, :VPAD], in_=vc[0])

            def load_bulk_rest():
                # remaining constants land during vproj(0)/attn(0,0)
                nc.scalar.dma_start(out=cos_sb[:], in_=cosT[:])
                nc.scalar.dma_start(out=sin_sb[:], in_=sinT[:])
                nc.scalar.dma_start(
                    out=mask_sb[:].rearrange("p (d c) -> p d c", c=NQ),
                    in_=masks[:].rearrange("d p c -> p d c"),
                )
                nc.scalar.dma_start(out=kT_sb[1][:, :CACHE], in_=kTc[1])
                nc.scalar.dma_start(out=v_sb[1][:, :VPAD], in_=vc[1])
            # rq_sb doubles as the q staging buffer (rope runs in place);
            # k stages directly into kT_sb[:, CACHE:].
            rq_sb = [constp.tile([128, S], bf16, tag=f"rq{h}", name=f"rq{h}")
                     for h in range(HPC)]
            onescol = constp.tile([128, 1], bf16, tag="onescol")
            nc.vector.memset(onescol[:], 1.0)
            # per-AR-chunk ssq partial tiles (separate tiles so a chunk's
            # readback doesn't pick up false whole-tile deps on later slices)
            ssq_c = []
            for ci, (j0, j1) in enumerate(AR_CHUNKS):
                w = (N_SLICES[j1 - 1][0] + N_SLICES[j1 - 1][1]
                     - N_SLICES[j0][0])
                ssq_c.append([
                    constp.tile([1, w], f32, tag=f"ssq_c{ci}_{ti}",
                                name=f"ssq_c{ci}_{ti}")
                    for ti in range(2)])
            chunk_of = {}
            for ci, (j0, j1) in enumerate(AR_CHUNKS):
                for j in range(j0, j1):
                    chunk_of[j] = ci
            # bf16 rsqrt rows used by rope (q row 0 / k row 0 separately)
            srow16 = [constp.tile([1, S], bf16, tag=f"srow16_{i}",
                                  name=f"srow16_{i}")
                      for i in range(2)]
            eps_col = constp.tile([1, 1], f32, tag="eps")
            nc.vector.memset(eps_col[:], EPS)

            def stage_dest(tname, m, qb, nn):
                if tname == "q":
                    return rq_sb[m][:, qb:qb + nn]
                return kT_sb[m][:, CACHE + qb:CACHE + qb + nn]

            def stream_x(qb, nn, consume):
                """DMA xT[:, qb:qb+nn] in 4-chunk groups; call consume(kc, rhs_ap)."""
                for g in range(KC // 4):
                    xs = xsp.tile([128, 4 * NQ], bf16, tag="xs", name="xs")
                    nc.sync.dma_start(
                        out=xs[:].rearrange("p (a n) -> p a n", n=NQ)[:, :, :nn],
                        in_=xT[g * 512:(g + 1) * 512, qb:qb + nn]
                            .rearrange("(a p) n -> p a n", p=128))
                    for kcl in range(4):
                        consume(g * 4 + kcl, xs[:, kcl * NQ:kcl * NQ + nn])

            def chunk_cols(ci):
                j0, j1 = AR_CHUNKS[ci]
                c0 = N_SLICES[j0][0]
                w = N_SLICES[j1 - 1][0] + N_SLICES[j1 - 1][1] - c0
                return c0, w, slice(c0, c0 + w)

            def fire_ar(ci):
                c0, w, sl = chunk_cols(ci)
                nc.gpsimd.dma_start(out=ssq_in_d[ci][:, :w],
                                    in_=ssq_c[ci][0][:])
                nc.gpsimd.dma_start(out=ssq_in_d[ci][:, w:],
                                    in_=ssq_c[ci][1][:])
                nc.gpsimd.collective_compute(
                    "AllGather", mybir.AluOpType.bypass, replica_groups=RG,
                    ins=[ssq_in_d[ci][:]], outs=[ssq_out_d[ci][:]])

            def recv_ar(ci):
                c0, w, sl = chunk_cols(ci)
                # gather-readback + local 8-row reduce, in <=1024-col pieces
                for ti in range(2):
                    t_ = ssq_c[ci][ti]
                    for p0 in range(0, w, 2 * NQ):
                        pw = min(2 * NQ, w - p0)
                        parts = workp.tile([N_CORES, 2 * NQ], f32,
                                           tag="ssqparts", bufs=1,
                                           name="ssq_parts")
                        nc.gpsimd.dma_start(
                            out=parts[:, :pw],
                            in_=ssq_out_d[ci][:, ti * w + p0:ti * w + p0 + pw])
                        nc.gpsimd.partition_all_reduce(
                            parts[:, :pw], parts[:, :pw], channels=N_CORES,
                            reduce_op=bass_isa.ReduceOp.add)
                        # srow = 1/sqrt(ssq/D + eps), then bf16 for rope
                        nc.scalar.activation(t_[:, p0:p0 + pw],
                                             parts[:1, :pw], Sqrt,
                                             scale=1.0 / D, bias=eps_col[:])
                    nc.vector.reciprocal_approx_fast(out=t_[:], in_=t_[:])
                    nc.vector.tensor_copy(srow16[ti][:, sl], t_[:])

            # ---- phase 1: merged q+k projection, all slices ----
            for jsl, (qb, nn) in enumerate(N_SLICES):
                pst = {t: [psac.tile([128, NQ], f32, tag="acc",
                                     name=f"proj_{t}{m}")
                           for m in range(HPC)] for t in ("q", "k")}

                def mm_proj(kc, rhs, pst=pst, nn=nn):
                    for t in ("q", "k"):
                        for m in range(HPC):
                            nc.tensor.matmul(
                                pst[t][m][:, :nn],
                                w_sb[t][:, kc * MF + m * 128:
                                        kc * MF + (m + 1) * 128],
                                rhs, start=(kc == 0), stop=(kc == KC - 1))

                stream_x(qb, nn, mm_proj)
                # ssq partials: stage bf16 (DVE), square the staged copy
                # (DVE, keeps the scalar queue free for DMA), ones-matmul
                # reduce on PE
                sqp = pssc.tile([128, 2 * NQ], f32, tag="scores", name="sqp")
                for ti, t in enumerate(("q", "k")):
                    for m in range(HPC):
                        # stage raw q/k as bf16 for post-AR in-place rope
                        st = stage_dest(t, m, qb, nn)
                        nc.vector.tensor_copy(st, pst[t][m][:, :nn])
                        q2 = workp.tile([128, NQ], bf16, tag="btmp")
                        nc.vector.tensor_tensor(q2[:, :nn], st, st, mult_op)
                        nc.tensor.matmul(sqp[:1, ti * NQ:ti * NQ + nn],
                                         onescol[:], q2[:, :nn],
                                         start=(m == 0), stop=(m == HPC - 1))
                    ci = chunk_of[jsl]
                    lo = qb - N_SLICES[AR_CHUNKS[ci][0]][0]
                    nc.vector.tensor_copy(ssq_c[ci][ti][:, lo:lo + nn],
                                          sqp[:1, ti * NQ:ti * NQ + nn])
                # fire the AllReduce as soon as its last slice is done
                for ci, (j0, j1) in enumerate(AR_CHUNKS):
                    if qb == N_SLICES[j1 - 1][0]:
                        fire_ar(ci)
                        if ci == 0:
                            load_bulk_first()
            load_bulk_rest()
            nc.scalar.dma_start(out=w_sb["v"][:], in_=wv[:])

            # ---------------- per-slice helpers ----------------
            def rope_j(j):
                qb, nn = N_SLICES[j]
                for m in range(HPC):
                    for ti, tname in enumerate(("q", "k")):
                        st = stage_dest(tname, m, qb, nn)
                        sh = workp.tile([128, NQ], bf16, tag="btmp")
                        nc.vector.stream_shuffle(sh[:, :nn], st, SWAP_MASK)
                        a = ftmp.tile([128, NQ], bf16, tag="btmp2",
                                      name="rope_a")
                        nc.vector.tensor_tensor(
                            a[:, :nn], st, cos_sb[:, qb:qb + nn], mult_op)
                        b = ftmp.tile([128, NQ], bf16, tag="btmp2",
                                      name="rope_b")
                        nc.vector.tensor_tensor(
                            b[:, :nn], sh[:, :nn], sin_sb[:, qb:qb + nn],
                            mult_op)
                        nc.vector.tensor_tensor(a[:, :nn], a[:, :nn],
                                                b[:, :nn], add_op)
                        srb = workp.tile([128, NQ], bf16, tag="srowb")
                        nc.gpsimd.partition_broadcast(
                            srb[:, :nn], srow16[ti][:, qb:qb + nn])
                        nc.vector.tensor_tensor(st, a[:, :nn], srb[:, :nn],
                                                mult_op)

            def vproj_j(j):
                qb, nn = N_SLICES[j]
                nst = (nn + 127) // 128
                xsg = []
                for g in range(KC // 4):
                    xs = xsp.tile([128, 4 * NQ], bf16, tag="xs", name="xsv")
                    nc.sync.dma_start(
                        out=xs[:].rearrange("p (a n) -> p a n", n=NQ)[:, :, :nn],
                        in_=xT[g * 512:(g + 1) * 512, qb:qb + nn]
                            .rearrange("(a p) n -> p a n", p=128))
                    xsg.append(xs)
                for s_ in range(nst):
                    sw = min(128, nn - s_ * 128)
                    pv = psac.tile([128, NQ], f32, tag="acc", name="pv_ps")
                    for g in range(KC // 4):
                        for kcl in range(4):
                            kc = g * 4 + kcl
                            nc.tensor.matmul(
                                pv[:sw, :MF],
                                xsg[g][:, kcl * NQ + s_ * 128:
                                       kcl * NQ + s_ * 128 + sw],
                                w_sb["v"][:, kc * MF:(kc + 1) * MF],
                                start=(kc == 0), stop=(kc == KC - 1))
                    st_glob = (qb + s_ * 128) // 128
                    for h in range(HPC):
                        nc.vector.tensor_copy(
                            v_sb[h][:sw, VPAD + st_glob * 128:
                                    VPAD + st_glob * 128 + 128],
                            pv[:sw, h * 128:(h + 1) * 128])

            scale = float(HD) ** -0.5
            GSZ = 8   # pairs per bf16 partial-sum group (16 k-tiles)

            def yproj(j):
                qb, nn = N_SLICES[j]
                py = [psac.tile([128, NQ], f32, tag="acc", name="py_ps")
                      for _ in range(HPC)]
                for g in range(KC // 4):
                    gt = xsp.tile([128, 4 * NQ], bf16, tag="xs", name="gt")
                    nc.sync.dma_start(
                        out=gt[:].rearrange("p (a n) -> p a n", n=NQ)[:, :, :nn],
                        in_=ag_out[j].rearrange("h r n -> (h r) n")
                            [g * 512:(g + 1) * 512, :]
                            .rearrange("(a p) n -> p a n", p=128))
                    for kcl in range(4):
                        kc = g * 4 + kcl
                        for m in range(HPC):
                            nc.tensor.matmul(
                                py[m][:, :nn],
                                w_sb["o"][:, kc * MF + m * 128:
                                          kc * MF + (m + 1) * 128],
                                gt[:, kcl * NQ:kcl * NQ + nn],
                                start=(kc == 0), stop=(kc == KC - 1))
                for m in range(HPC):
                    ys = ftmp.tile([128, NQ], f32, tag="f32tmp", name="ys")
                    nc.vector.tensor_copy(ys[:, :nn], py[m][:, :nn])
                    nc.sync.dma_start(
                        out=yT[m * 128:(m + 1) * 128, qb:qb + nn],
                        in_=ys[:, :nn])

            def attn_jh(j, h):
                qb, nn = N_SLICES[j]
                # k-tile list: (col0 in kT_sb, kt, vcol0, mask_off)
                tiles = []
                for ct in range(CTILES):
                    kt = min(128, CACHE - ct * 128)
                    tiles.append((ct * 128, kt, ct * 128, None))
                for t in range(NTILES):
                    kb = t * 128
                    if kb > qb + nn - 1:
                        continue
                    kt = min(128, S - kb)
                    moff = (kb - qb) if (kb + kt - 1) > qb else None
                    tiles.append((CACHE + kb, kt, VPAD + kb, moff))
                # pair up consecutive full tiles to halve per-instruction
                # overheads on ACT/DVE
                pairs = []
                i = 0
                while i < len(tiles):
                    if (i + 1 < len(tiles) and tiles[i][1] == 128
                            and tiles[i + 1][1] == 128):
                        pairs.append((tiles[i], tiles[i + 1]))
                        i += 2
                    else:
                        pairs.append((tiles[i],))
                        i += 1
                out_ps = psac.tile([128, NQ], f32, tag="acc", name="out_ps")
                pacc = attnp.tile([128, 2 * NQ], f32, tag="pacc", bufs=2)
                rq_slice = rq_sb[h][:, qb:qb + nn]
                nidx = 0
                nlast = len(tiles) - 1
                gacc = None
                gcount = 0
                pacc_init = False

                def flush(nn=nn):
                    nonlocal gacc, gcount, pacc_init
                    if gacc is None:
                        return
                    gv = gacc[:].rearrange(
                        "p (a n) -> p a n", n=NQ)[:, :, :nn]
                    pv_ = pacc[:].rearrange(
                        "p (a n) -> p a n", n=NQ)[:, :, :nn]
                    if pacc_init:
                        nc.vector.tensor_tensor(pv_, pv_, gv, add_op)
                    else:
                        nc.vector.tensor_copy(pv_, gv)
                    gacc = None
                    gcount = 0
                    pacc_init = True

                for pair in pairs:
                    full_pair = len(pair) == 2
                    sc = pssc.tile([128, 2 * NQ], f32, tag="scores")
                    for half, (c0, kt, vcol, moff) in enumerate(pair):
                        nc.tensor.matmul(
                            sc[:kt, half * NQ:half * NQ + nn],
                            kT_sb[h][:, c0:c0 + kt],
                            rq_slice, start=True, stop=True)
                    kt0 = pair[0][1]
                    # exp of a group's first full pair writes the group
                    # accumulator directly (saves a DVE copy per group)
                    new_group = full_pair and gacc is None
                    if new_group:
                        gacc = attnp.tile([128, 2 * NQ], bf16,
                                          tag="gacc", bufs=2)
                        pt = gacc
                        gcount = 1
                    else:
                        pt = ptp.tile([128, 2 * NQ], bf16, tag="pT")
                    if full_pair:
                        nc.scalar.activation(
                            pt[:].rearrange("p (a n) -> p a n",
                                            n=NQ)[:, :, :nn],
                            sc[:].rearrange("p (a n) -> p a n",
                                            n=NQ)[:, :, :nn],
                            Exp, scale=scale)
                    else:
                        nc.scalar.activation(pt[:kt0, :nn],
                                             sc[:kt0, :nn], Exp,
                                             scale=scale)
                    for half, (c0, kt, vcol, moff) in enumerate(pair):
                        if moff is not None:
                            mi = moff // 128
                            nc.vector.tensor_tensor(
                                pt[:kt, half * NQ:half * NQ + nn],
                                pt[:kt, half * NQ:half * NQ + nn],
                                mask_sb[:kt, mi * NQ:mi * NQ + nn],
                                mult_op)
                    # denominator accumulation: bf16 groups of GSZ pairs,
                    # folded into fp32 pacc; odd tiles direct
                    if full_pair:
                        if not new_group:
                            nc.vector.tensor_tensor(
                                gacc[:].rearrange("p (a n) -> p a n",
                                                  n=NQ)[:, :, :nn],
                                gacc[:].rearrange("p (a n) -> p a n",
                                                  n=NQ)[:, :, :nn],
                                pt[:].rearrange("p (a n) -> p a n",
                                                n=NQ)[:, :, :nn],
                                add_op)
                            gcount += 1
                        if gcount == GSZ:
                            flush()
                    else:
                        flush()
                        if pacc_init:
                            nc.vector.tensor_tensor(
                                pacc[:kt0, :nn], pacc[:kt0, :nn],
                                pt[:kt0, :nn], add_op)
                        else:
                            nc.vector.tensor_copy(pacc[:kt0, :nn],
                                                  pt[:kt0, :nn])
                            pacc_init = True
                    for half, (c0, kt, vcol, moff) in enumerate(pair):
                        nc.tensor.matmul(
                            out_ps[:, :nn],
                            v_sb[h][:kt, vcol:vcol + 128],
                            pt[:kt, half * NQ:half * NQ + nn],
                            start=(nidx == 0), stop=(nidx == nlast))
                        nidx += 1
                flush()
                # fold the two halves, reduce over partitions, reciprocal
                nc.vector.tensor_tensor(pacc[:, :nn], pacc[:, :nn],
                                        pacc[:, NQ:NQ + nn], add_op)
                recb = attnp.tile([128, NQ], f32, tag="recb", bufs=2)
                nc.gpsimd.partition_all_reduce(
                    recb[:, :nn], pacc[:, :nn], channels=128,
                    reduce_op=bass_isa.ReduceOp.add)
                nc.vector.reciprocal_approx_fast(out=recb[:, :nn],
                                                 in_=recb[:, :nn])
                onorm = attnp.tile([128, NQ], bf16, tag="onorm", bufs=2)
                nc.vector.tensor_tensor(onorm[:, :nn], out_ps[:, :nn],
                                        recb[:, :nn], mult_op)
                nc.gpsimd.dma_start(out=ag_in[j][h][:, :nn],
                                    in_=onorm[:, :nn])
                nc.gpsimd.collective_compute(
                    "AllGather", mybir.AluOpType.bypass, replica_groups=RG,
                    ins=[ag_in[j][h]], outs=[ag_out[j][h]])

            # ---- phase 2: per-slice vproj / rope / attention / yproj ----
            # The last two slices interleave head-wise so the final
            # AllGathers on the serial CC queue are the small (80-wide)
            # ones and the 512-wide ones overlap attention compute.
            nc.scalar.dma_start(out=w_sb["o"][:], in_=wo[:])
            recv_at = {j: ci for ci, j in AR_RECV_AT.items()}
            for j in range(NJ - 2):
                if j in recv_at:
                    recv_ar(recv_at[j])
                rope_j(j)
                vproj_j(j)
                attn_jh(j, 0)
                attn_jh(j, 1)
                if j >= 1:
                    yproj(j - 1)
            for j in (NJ - 2, NJ - 1):
                if j in recv_at:
                    recv_ar(recv_at[j])
                rope_j(j)
                vproj_j(j)
            attn_jh(NJ - 2, 0)
            yproj(NJ - 3)
            attn_jh(NJ - 2, 1)
            attn_jh(NJ - 1, 0)
            attn_jh(NJ - 1, 1)
            yproj(NJ - 2)
            yproj(NJ - 1)
    nc.compile()
    return nc


def get_program():
    if "nc" not in _prog_cache:
        _prog_cache["nc"] = build_program()
    return _prog_cache["nc"]


def prep_inputs(x, freqs, k_cache, v_cache, Wq, bq, Wk, bk, Wv, bv, Wo, bo,
                gq, gk, current_start):
    """Host-side sharding/layout. Returns per-core in_maps."""
    cs = int(current_start)
    x = np.asarray(x, dtype=np.float32)
    xT = np.ascontiguousarray(x[0].T).astype(BF)           # [D, S]
    freqs = np.asarray(freqs, dtype=np.float32)
    csl = freqs[cs:cs + S, :HD // 2]                       # [S, 64]
    snl = freqs[cs:cs + S, HD // 2:]                       # [S, 64]
    cosT = np.empty((128, S), np.float32)
    sinT = np.empty((128, S), np.float32)
    cosT[0::2] = csl.T
    cosT[1::2] = csl.T
    sinT[0::2] = -snl.T
    sinT[1::2] = snl.T
    cosT = cosT.astype(BF)
    sinT = sinT.astype(BF)
    # spec guarantees zero biases and unit gains; the device program
    # relies on that (cheap to add back via K=1 bias matmuls if needed)
    for b in (bq, bk, bv, bo):
        assert not np.any(np.asarray(b)), "nonzero bias unsupported"
    for g in (gq, gk):
        assert np.all(np.asarray(g) == 1.0), "non-unit gain unsupported"
    # masks: multiplicative {0,1}, mask_d[r, c] = 1 if c >= r + d
    masks = np.zeros((4, 128, NQ), np.float32)
    r = np.arange(128)[:, None]
    c = np.arange(NQ)[None, :]
    for di, d in enumerate((0, 128, 256, 384)):
        masks[di] = (c >= r + d).astype(np.float32)
    masks = masks.astype(BF)

    k_cache = np.asarray(k_cache, np.float32)
    v_cache = np.asarray(v_cache, np.float32)

    def wlayout(W, sl, perm=None):
        wt = np.ascontiguousarray(np.asarray(W, np.float32)[sl].T)  # [D, MF]
        if perm is not None:
            wt = wt[perm]
        return np.ascontiguousarray(
            wt.reshape(KC, 128, MF).transpose(1, 0, 2)
            .reshape(128, KC * MF)).astype(BF)

    in_maps = []
    for core in range(N_CORES):
        h0 = core * HPC
        sl = slice(core * MF, (core + 1) * MF)
        kTc = np.ascontiguousarray(
            np.transpose(k_cache[:, h0:h0 + HPC, :], (1, 2, 0))).astype(BF)
        vpad = np.zeros((HPC, 128, VPAD), BF)
        for h in range(HPC):
            vt = np.zeros((VPAD, HD), np.float32)
            vt[:CACHE] = v_cache[:, h0 + h, :]
            vpad[h] = np.ascontiguousarray(
                vt.reshape(CTILES, 128, HD).transpose(1, 0, 2)
                .reshape(128, VPAD)).astype(BF)
        in_maps.append({
            "xT": xT,
            "wq": wlayout(Wq, sl),
            "wk": wlayout(Wk, sl),
            "wv": wlayout(Wv, sl),
            "wo": wlayout(Wo, sl, perm=AG_PERM),
            "cosT": cosT,
            "sinT": sinT,
            "kTc": kTc,
            "vc": vpad,
            "masks": masks,
        })
    return in_maps


def assemble_output(results):
    cols = [np.asarray(r["yT"], np.float32).T for r in results]  # [S, MF] each
    return np.ascontiguousarray(np.concatenate(cols, axis=1))[None]


def run(inputs, trace=False):
    nc = get_program()
    in_maps = prep_inputs(**inputs)
    r = run_bass_kernel_spmd(nc, in_maps, core_ids=list(range(N_CORES)),
                             trace=trace)
    return assemble_output(r.results), r


def kernel(**inputs):
    out, _ = run(inputs, trace=False)
    return out
